# revision 1
# baseline (speedup 1.0000x reference)
"""Causal self-attention (RoPE, 16 heads) on 8 Trainium2 NeuronCores.

Sharding: core s -> (batch b = s//2, head-half g = s%2). Each core computes
qkv = x_b @ w_attn[:, heads g], RoPE, causal SDPA for its 8 heads, and a
partial y_local @ w_proj[rows g] -> [T, C]. Host sums the two partials per
batch (row-parallel Megatron unshard).

All matmuls run in float32r (TF32-like, full PE rate at N=512;
measured rel err ~2e-4 end-to-end).
"""

import sys

sys.path.insert(0, "/opt/trn_rl_repo")

import numpy as np

import concourse.bacc as bacc
import concourse.mybir as mybir
import concourse.tile as tile
from concourse.masks import make_identity

P = 128
D = 128
F32 = mybir.dt.float32
F32R = mybir.dt.float32r
EXP = mybir.ActivationFunctionType.Exp

NUM_HEADS = 16
ROPE_THETA = 10000.0


def build_nc(
    T=2048,
    CIN=2048,
    HL=8,
    COUT=2048,
    *,
    w_bufs=2,
    acc_bufs=3,
    e_bufs=8,
    st_bufs=4,
    att_bufs=4,
    head_order=True,
    denom=True,
    stages=(1, 2, 3),
    yps_bufs=2,
    dps_bufs=1,
    bps_bufs=1,
    xt_split=False,
    tps_bufs=4,
    proj_split=1,
    rope_bufs=3,
    vout_bufs=4,
    o_bufs=3,
    ps3_bufs=3,
    vdirect=False,
):
    """Build the per-core Bass program (identical on all cores)."""
    CL = HL * D          # local qkv width per section (1024)
    NQK = 2 * CL // P    # q|k n-tiles (16)
    NV = CL // P         # v n-tiles (8) == HL
    CC = CIN // P        # contraction chunks (16)
    TB = T // 512        # 512-wide t blocks (4)
    TC = T // P          # 128-wide t chunks (16)
    NB = COUT // 512     # output col blocks (4)
    SCALE = 1.0 / float(np.sqrt(D))

    nc = bacc.Bacc("TRN2", target_bir_lowering=False, debug=False)

    xT_d = nc.dram_tensor("xT", [CIN, T], F32, kind="ExternalInput").ap()
    w_d = nc.dram_tensor("w", [CIN, 3 * CL], F32, kind="ExternalInput").ap()
    wp_d = nc.dram_tensor("wp", [CL, COUT], F32, kind="ExternalInput").ap()
    cosT_d = nc.dram_tensor("cosT", [D, T], F32, kind="ExternalInput").ap()
    sinT_d = nc.dram_tensor("sinT", [D, T], F32, kind="ExternalInput").ap()
    mask_d = nc.dram_tensor("mask", [4, P, 512], F32, kind="ExternalInput").ap()
    out_d = nc.dram_tensor("out", [T, COUT], F32, kind="ExternalOutput").ap()

    qkT_i = nc.dram_tensor("qkT_i", [2 * CL, T], F32R).ap()
    v_i = nc.dram_tensor("v_i", [HL, TC, P, P], F32R).ap()

    with tile.TileContext(nc) as tc:
        with (
            tc.tile_pool(name="const", bufs=1) as cp,
            tc.tile_pool(name="cps", bufs=1, space="PSUM") as _cps,
        ):
            ident_f = cp.tile([P, P], F32)
            make_identity(nc, ident_f)
            ident = cp.tile([P, P], F32R)
            nc.vector.tensor_copy(ident[:], ident_f[:])
            ones_cf = cp.tile([P, 1], F32)
            nc.vector.memset(ones_cf[:], 1.0)
            ones_col = cp.tile([P, 1], F32R)
            nc.vector.tensor_copy(ones_col[:], ones_cf[:])
            ones_rf = cp.tile([1, P], F32)
            nc.vector.memset(ones_rf[:], 1.0)
            ones_row = cp.tile([1, P], F32R)
            nc.vector.tensor_copy(ones_row[:], ones_rf[:])
            # ---------------- stage 1: qkvT = (x @ w)^T tiles + RoPE / V transpose
            with tc.tile_pool(name="xt", bufs=1) as xtp:
                xT = xtp.tile([P, CC, T], F32R)
                if xt_split:
                    xr = xT_d.bitcast(F32R).rearrange("(c p) t -> p c t", p=P)
                    for c in range(CC):
                        nc.sync.dma_start(xT[:, c], xr[:, c])
                else:
                    nc.sync.dma_start(
                        xT[:], xT_d.bitcast(F32R).rearrange("(c p) t -> p c t", p=P)
                    )
                with (
                    tc.tile_pool(name="ropetab", bufs=1) as rtp,
                    tc.tile_pool(name="w1", bufs=w_bufs) as wpool,
                    tc.tile_pool(name="rope", bufs=rope_bufs) as rp,
                    tc.tile_pool(name="vout", bufs=vout_bufs) as vp,
                    tc.tile_pool(name="ps1", bufs=acc_bufs, space="PSUM") as ps1,
                    tc.tile_pool(name="tps", bufs=tps_bufs, space="PSUM") as tps,
                ):
                    cosT = rtp.tile([D, T], F32)
                    sinT = rtp.tile([D, T], F32)
                    nc.sync.dma_start(cosT[:], cosT_d[:])
                    nc.sync.dma_start(sinT[:], sinT_d[:])

                    if head_order:
                        # q_h, k_h, v_h grouped per head so attention for
                        # head h can start after 3*(h+1)/24 of stage 1
                        nt_order = [
                            x for h in range(HL) for x in (h, HL + h, 2 * HL + h)
                        ]
                    else:
                        nt_order = list(range(NQK + NV))
                    for nt in (nt_order if 1 in stages else []):
                        w_sb = wpool.tile([P, CC, P], F32R, name="w_sb")
                        nc.sync.dma_start(
                            w_sb[:],
                            w_d.bitcast(F32R)[:, nt * P : (nt + 1) * P].rearrange(
                                "(c p) n -> p c n", p=P
                            ),
                        )
                        for tb in range(TB):
                            acc = ps1.tile([P, 512], F32, name="acc")
                            for c in range(CC):
                                nc.tensor.matmul(
                                    acc[:],
                                    w_sb[:, c],
                                    xT[:, c, tb * 512 : (tb + 1) * 512],
                                    start=(c == 0),
                                    stop=(c == CC - 1),
                                )
                            if nt >= NQK:
                                h = nt - NQK
                                vT_sb = vp.tile([P, 512], F32R, name="vT_sb")
                                nc.vector.tensor_copy(vT_sb[:], acc[:])
                                for q4 in range(4):
                                    tp = tps.tile([P, P], F32R, name="tp")
                                    nc.tensor.transpose(
                                        tp[:],
                                        vT_sb[:, q4 * P : (q4 + 1) * P],
                                        ident[:],
                                    )
                                    if vdirect:
                                        nc.sync.dma_start(
                                            v_i[h, tb * 4 + q4], tp[:]
                                        )
                                    else:
                                        v_nat = vp.tile([P, P], F32R, name="v_nat")
                                        nc.vector.tensor_copy(v_nat[:], tp[:])
                                        nc.sync.dma_start(
                                            v_i[h, tb * 4 + q4], v_nat[:]
                                        )
                                continue
                            # RoPE: q' = q*cos + rot(q)*sin, rot = [-q_hi, q_lo]
                            raw = rp.tile([P, 512], F32, name="raw")
                            nc.scalar.copy(raw[:], acc[:])
                            rot = rp.tile([P, 512], F32, name="rot")
                            nc.scalar.mul(rot[0:64, :], raw[64:128, :], -1.0)
                            nc.scalar.copy(rot[64:128, :], raw[0:64, :])
                            cs = slice(tb * 512, (tb + 1) * 512)
                            nc.vector.tensor_mul(raw[:], raw[:], cosT[:, cs])
                            nc.vector.tensor_mul(rot[:], rot[:], sinT[:, cs])
                            qk_out = rp.tile([P, 512], F32R, name="qk_out")
                            nc.vector.tensor_add(qk_out[:], raw[:], rot[:])
                            nc.sync.dma_start(
                                qkT_i[nt * P : (nt + 1) * P, cs], qk_out[:]
                            )

            # ---------------- stage 2: causal attention per head -> yT in SBUF
            with tc.tile_pool(name="yt", bufs=1) as ytp:
                yT = ytp.tile([P, HL, T], F32R)
                with (
                    tc.tile_pool(name="maskp", bufs=1) as mp,
                    tc.tile_pool(name="att", bufs=att_bufs) as ap,
                    tc.tile_pool(name="esb", bufs=e_bufs) as ep,
                    tc.tile_pool(name="div", bufs=2) as dp,
                    tc.tile_pool(name="ps2", bufs=1, space="PSUM") as ps2,
                    tc.tile_pool(name="stps", bufs=st_bufs, space="PSUM") as stps,
                ):
                    masks = mp.tile([P, 4, 512], F32R)
                    nc.sync.dma_start(
                        masks[:], mask_d.bitcast(F32R).rearrange("j p f -> p j f")
                    )
                    for h in (range(HL) if 2 in stages else []):
                        qT_h = ap.tile([P, T], F32R, name="qT_h")
                        nc.sync.dma_start(qT_h[:], qkT_i[h * P : (h + 1) * P, :])
                        kT_h = ap.tile([P, T], F32R, name="kT_h")
                        nc.sync.dma_start(
                            kT_h[:], qkT_i[CL + h * P : CL + (h + 1) * P, :]
                        )
                        V_h = ap.tile([P, TC, P], F32R, name="V_h")
                        nc.sync.dma_start(V_h[:], v_i[h].rearrange("c p d -> p c d"))
                        for b in range(TB):
                            nch = 4 * (b + 1)
                            bs = slice(b * 512, (b + 1) * 512)
                            yps = ps2.tile(
                                [P, 512], F32, name="yps", tag="yps", bufs=yps_bufs
                            )
                            dps = ps2.tile(
                                [1, 512], F32, name="dps", tag="dps", bufs=dps_bufs
                            )
                            for c in range(nch):
                                st = stps.tile([P, 512], F32, name="st")
                                nc.tensor.matmul(
                                    st[:],
                                    kT_h[:, c * P : (c + 1) * P],
                                    qT_h[:, bs],
                                    start=True,
                                    stop=True,
                                )
                                e_sb = ep.tile([P, 512], F32R, name="e_sb")
                                nc.scalar.activation(e_sb[:], st[:], EXP, scale=SCALE)
                                j = c - (nch - 4)
                                if j >= 0:
                                    nc.vector.tensor_mul(e_sb[:], e_sb[:], masks[:, j])
                                nc.tensor.matmul(
                                    yps[:], V_h[:, c], e_sb[:],
                                    start=(c == 0), stop=(c == nch - 1),
                                )
                                if denom:
                                    nc.tensor.matmul(
                                        dps[:], ones_col[:], e_sb[:],
                                        start=(c == 0), stop=(c == nch - 1),
                                    )
                            recip = dp.tile([1, 512], F32R, name="recip")
                            with nc.allow_low_precision(reason="fp32r recip"):
                                nc.vector.reciprocal(recip[:], dps[:])
                            bps = ps2.tile(
                                [P, 512], F32, name="bps", tag="bps", bufs=bps_bufs
                            )
                            nc.tensor.matmul(
                                bps[:], ones_row[:], recip[:], start=True, stop=True
                            )
                            bc = dp.tile([P, 512], F32, name="bc")
                            nc.scalar.copy(bc[:], bps[:])
                            nc.vector.tensor_mul(yT[:, h, bs], yps[:], bc[:])

                # ------------ stage 3: partial out = yT.T @ wp
                with (
                    tc.tile_pool(name="wpp", bufs=1) as wpp,
                    tc.tile_pool(name="o", bufs=o_bufs) as op,
                    tc.tile_pool(name="ps3", bufs=ps3_bufs, space="PSUM") as ps3,
                ):
                    wp_sb = wpp.tile([P, HL, COUT], F32R)
                    nc.sync.dma_start(
                        wp_sb[:], wp_d.bitcast(F32R).rearrange("(h p) n -> p h n", p=P)
                    )
                    HG = HL // proj_split
                    for half in range(proj_split if 3 in stages else 0):
                        hs = range(half * HG, (half + 1) * HG)
                        for tt in range(TC):
                            for nb in range(NB):
                                ps3t = ps3.tile([P, 512], F32, name="ps3t")
                                for i, h in enumerate(hs):
                                    nc.tensor.matmul(
                                        ps3t[:],
                                        yT[:, h, tt * P : (tt + 1) * P],
                                        wp_sb[:, h, nb * 512 : (nb + 1) * 512],
                                        start=(i == 0),
                                        stop=(i == HG - 1),
                                    )
                                o_sb = op.tile([P, 512], F32, name="o_sb")
                                nc.scalar.copy(o_sb[:], ps3t[:])
                                dst = out_d[
                                    tt * P : (tt + 1) * P, nb * 512 : (nb + 1) * 512
                                ]
                                if half == 0:
                                    nc.sync.dma_start(dst, o_sb[:])
                                else:
                                    nc.gpsimd.dma_start(
                                        dst, o_sb[:], accum_op=mybir.AluOpType.add
                                    )

    nc.compile()
    return nc


def _rope_tables_T(T, head_dim):
    half = head_dim // 2
    inv_freq = 1.0 / (ROPE_THETA ** (np.arange(0, half, dtype=np.float64) / half))
    ang = np.arange(T, dtype=np.float64)[:, None] * inv_freq[None, :]  # [T, half]
    cos = np.concatenate([np.cos(ang), np.cos(ang)], axis=-1)  # [T, D]
    sin = np.concatenate([np.sin(ang), np.sin(ang)], axis=-1)
    return (
        np.ascontiguousarray(cos.T.astype(np.float32)),
        np.ascontiguousarray(sin.T.astype(np.float32)),
    )


def _make_masks():
    m = np.zeros((4, P, 512), dtype=np.float32)
    f = np.arange(512)[None, :]
    p = np.arange(P)[:, None]
    for j in range(4):
        m[j] = (f >= p + j * 128).astype(np.float32)
    return m


_NC_CACHE = {}


def _get_nc(T, CIN, HL, COUT):
    key = (T, CIN, HL, COUT)
    if key not in _NC_CACHE:
        _NC_CACHE[key] = build_nc(T, CIN, HL, COUT)
    return _NC_CACHE[key]


def make_in_maps(x, w_attn, w_proj):
    x = np.asarray(x)
    w_attn = np.asarray(w_attn)
    w_proj = np.asarray(w_proj)
    B, T, C = x.shape
    HL = NUM_HEADS // 2  # 8 heads per core
    CL = HL * D  # 1024

    cosT, sinT = _rope_tables_T(T, D)
    masks = _make_masks()

    in_maps = []
    for s in range(8):
        b, g = s // 2, s % 2
        w_shard = np.concatenate(
            [
                w_attn[:, g * CL : (g + 1) * CL],
                w_attn[:, C + g * CL : C + (g + 1) * CL],
                w_attn[:, 2 * C + g * CL : 2 * C + (g + 1) * CL],
            ],
            axis=1,
        ).astype(np.float32)
        in_maps.append(
            {
                "xT": np.ascontiguousarray(x[b].T).astype(np.float32),
                "w": np.ascontiguousarray(w_shard),
                "wp": np.ascontiguousarray(w_proj[g * CL : (g + 1) * CL, :]).astype(
                    np.float32
                ),
                "cosT": cosT,
                "sinT": sinT,
                "mask": masks,
            }
        )
    return in_maps


def combine(results, x_shape):
    B, T, C = x_shape
    out = np.empty((B, T, C), dtype=np.float32)
    for b in range(B):
        out[b] = results[2 * b]["out"] + results[2 * b + 1]["out"]
    return out


def kernel(x, w_attn, w_proj):
    from concourse.bass_utils import run_bass_kernel_spmd

    x = np.asarray(x)
    B, T, C = x.shape  # 4, 2048, 2048
    HL = NUM_HEADS // 2

    nc = _get_nc(T, C, HL, C)
    in_maps = make_in_maps(x, w_attn, w_proj)
    res = run_bass_kernel_spmd(nc, in_maps, list(range(8)))
    return combine(res.results, (B, T, C))



# revision 3
# speedup vs baseline: 2.5841x; 2.5841x over previous
"""Causal self-attention (RoPE, 16 heads) on 8 Trainium2 NeuronCores — fused.

Sharding: core s -> (batch b = s//2, head-half g = s%2). Each core computes
qkv = x_b @ w_attn[:, heads g], RoPE, causal SDPA for its 8 heads, and a
partial y_local @ w_proj[rows g] -> [T, C]. Host sums the two partials per
batch (row-parallel Megatron unshard).

v2: single fused pipeline, everything SBUF-resident in bf16:
 - x, w_attn, w_proj, cos/sin, masks converted to bf16 on host.
 - per head h: qkv piece per 512-col t-block (q,k via w-stationary matmul +
   RoPE; V directly in [t,d] layout via x-stationary matmul, no transpose),
   interleaved at block granularity with attention of head h-1 so the PE
   never waits on the ACT-engine exp chain.
 - softmax denominator: DVE accumulates exp chunks elementwise; one
   ones-matmul per (head, block) reduces across partitions + broadcasts.
 - attention of head 7 interleaves with the output projection; w_proj is
   prefetched during the head loop.
"""

import sys

sys.path.insert(0, "/opt/trn_rl_repo")

import numpy as np

import concourse.bacc as bacc
import concourse.mybir as mybir
import concourse.tile as tile

P = 128
D = 128
F32 = mybir.dt.float32
BF16 = mybir.dt.bfloat16
EXP = mybir.ActivationFunctionType.Exp

NUM_HEADS = 16
ROPE_THETA = 10000.0


def build_nc(
    T=2048,
    CIN=2048,
    HL=8,
    COUT=2048,
    *,
    w_bufs=2,
    acc_bufs=2,
    st_bufs=3,
    yps_bufs=2,
    vps_bufs=1,
    e_bufs=4,
    ds_bufs=2,
    rc_bufs=1,
    rope_bufs=2,
    qk_bufs=2,
    v_bufs=2,
    o_bufs=3,
):
    CC = CIN // P        # contraction chunks (16)
    TB = T // 512        # 512-wide t blocks (4)
    TC = T // P          # 128-wide t chunks (16)
    NB = COUT // 512     # output col blocks (4)
    SCALE = 1.0 / float(np.sqrt(D))

    nc = bacc.Bacc("TRN2", target_bir_lowering=False, debug=False)

    xT_d = nc.dram_tensor("xT", [CIN, T], BF16, kind="ExternalInput").ap()
    w_d = nc.dram_tensor("w", [HL, 3, P, CIN // P * D], BF16, kind="ExternalInput").ap()
    wp_d = nc.dram_tensor("wp", [HL * D, COUT], BF16, kind="ExternalInput").ap()
    cosT_d = nc.dram_tensor("cosT", [D, T], BF16, kind="ExternalInput").ap()
    sinT_d = nc.dram_tensor("sinT", [D, T], BF16, kind="ExternalInput").ap()
    mask_d = nc.dram_tensor("mask", [P, 896], BF16, kind="ExternalInput").ap()
    out_d = nc.dram_tensor("out", [T, COUT], BF16, kind="ExternalOutput").ap()

    with tile.TileContext(nc) as tc:
        with (
            tc.tile_pool(name="const", bufs=1) as cp,
            tc.tile_pool(name="xt", bufs=1) as xtp,
            tc.tile_pool(name="ropetab", bufs=1) as rtp,
            tc.tile_pool(name="maskp", bufs=1) as mp,
            tc.tile_pool(name="yt", bufs=1) as ytp,
            tc.tile_pool(name="wpp", bufs=1) as wpp,
            tc.tile_pool(name="w1", bufs=w_bufs) as wpool,
            tc.tile_pool(name="rope", bufs=rope_bufs) as rp,
            tc.tile_pool(name="qk", bufs=qk_bufs) as qkp,
            tc.tile_pool(name="vpool", bufs=v_bufs) as vp,
            tc.tile_pool(name="esb", bufs=e_bufs) as ep,
            tc.tile_pool(name="dsp", bufs=ds_bufs) as dsp,
            tc.tile_pool(name="rcp", bufs=rc_bufs) as rcp,
            tc.tile_pool(name="o", bufs=o_bufs) as op,
            tc.tile_pool(name="ps_acc", bufs=acc_bufs, space="PSUM") as accp,
            tc.tile_pool(name="ps_v", bufs=vps_bufs, space="PSUM") as vpsp,
            tc.tile_pool(name="ps_st", bufs=st_bufs, space="PSUM") as stp,
            tc.tile_pool(name="ps_y", bufs=yps_bufs, space="PSUM") as ypsp,
        ):
            # constants
            ones_bf = cp.tile([P, P], BF16)
            nc.vector.memset(ones_bf[:], 1.0)

            # resident tensors
            xT = xtp.tile([P, CC, T], BF16)
            cosT = rtp.tile([D, T], BF16)
            sinT = rtp.tile([D, T], BF16)
            masks = mp.tile([P, 896], BF16)
            yT = ytp.tile([P, HL, T], BF16)
            wp_sb = wpp.tile([P, HL, COUT], BF16)

            # --- startup DMAs (SP queue order matters) ---
            xr = xT_d.rearrange("(c p) t -> p c t", p=P)
            w_tiles = {}

            def load_w(h):
                w_f = wpool.tile([P, 3 * CC * D], BF16, name="w_h")
                w_h = w_f.rearrange("p (j c d) -> p j c d", j=3, c=CC)
                wr = w_d[h].rearrange("j p k -> p j k")
                if h == 0:
                    w_tiles[h] = (w_h, w_f, wr)  # DMAs interleaved below
                else:
                    nc.sync.dma_start(
                        w_f.rearrange("p (j k) -> p j k", j=3)[:], wr
                    )
                    w_tiles[h] = w_h

            load_w(0)
            w_h0, w_f0, wr0 = w_tiles[0]
            w_tiles[0] = w_h0
            KD = CC * D
            nc.sync.dma_start(xT[:, :, 0:256], xr[:, :, 0:256])
            nc.sync.dma_start(w_f0[:, 0:KD], wr0[:, 0])  # w_q0
            nc.sync.dma_start(w_f0[:, KD : 2 * KD], wr0[:, 1])  # w_k0
            nc.sync.dma_start(w_f0[:, 2 * KD :], wr0[:, 2])  # w_v0
            nc.sync.dma_start(xT[:, :, 256:512], xr[:, :, 256:512])
            nc.sync.dma_start(xT[:, :, 512:768], xr[:, :, 512:768])
            nc.sync.dma_start(xT[:, :, 768:1024], xr[:, :, 768:1024])
            nc.sync.dma_start(cosT[:], cosT_d[:])
            nc.sync.dma_start(sinT[:], sinT_d[:])
            for tb in range(2, TB):
                nc.sync.dma_start(
                    xT[:, :, tb * 512 : (tb + 1) * 512],
                    xr[:, :, tb * 512 : (tb + 1) * 512],
                )
            nc.sync.dma_start(masks[:], mask_d[:])

            qk_tiles = {}
            v_tiles = {}

            def qkv_piece(h, b):
                """q_h, k_h for t-block b (with RoPE) + V_h t-chunks 4b..4b+3."""
                w_h = w_tiles[h]
                bs = slice(b * 512, (b + 1) * 512)
                if b == 0:
                    qT_h = qkp.tile([P, T], BF16, name="qT_h")
                    kT_h = qkp.tile([P, T], BF16, name="kT_h")
                    V_h = vp.tile([P, TC, P], BF16, name="V_h")
                    qk_tiles[h] = (qT_h, kT_h)
                    v_tiles[h] = V_h
                qT_h, kT_h = qk_tiles[h]
                V_h = v_tiles[h]
                # the very first piece works in 256-col halves so compute can
                # start after half of the first xT chunk has landed
                halves = (
                    [(0, 256), (256, 512)] if (h == 0 and b == 0) else [(0, 512)]
                )
                for lo, hi in halves:
                    w_ = hi - lo
                    hs = slice(b * 512 + lo, b * 512 + hi)
                    for which, dst in ((0, qT_h), (1, kT_h)):
                        acc = accp.tile([P, 512], F32, name="acc")
                        for c in range(CC):
                            nc.tensor.matmul(
                                acc[:, 0:w_],
                                w_h[:, which, c],
                                xT[:, c, hs],
                                start=(c == 0),
                                stop=(c == CC - 1),
                            )
                        # RoPE: q' = q*cos + rot(q)*sin, rot = [-q_hi, q_lo]
                        raw = rp.tile([P, 512], F32, name="raw")
                        nc.scalar.copy(raw[:, 0:w_], acc[:, 0:w_])
                        rot = rp.tile([P, 512], F32, name="rot")
                        nc.scalar.mul(rot[0:64, 0:w_], raw[64:128, 0:w_], -1.0)
                        nc.scalar.copy(rot[64:128, 0:w_], raw[0:64, 0:w_])
                        nc.vector.tensor_mul(raw[:, 0:w_], raw[:, 0:w_], cosT[:, hs])
                        nc.vector.tensor_mul(rot[:, 0:w_], rot[:, 0:w_], sinT[:, hs])
                        nc.vector.tensor_add(dst[:, hs], raw[:, 0:w_], rot[:, 0:w_])
                    # V in [t, d] layout directly: x-stationary matmul
                    vps = vpsp.tile([P, 4, P], F32, name="vps")
                    t4s = range(lo // P, hi // P)
                    for t4 in t4s:
                        tt = 4 * b + t4
                        for c in range(CC):
                            nc.tensor.matmul(
                                vps[:, t4],
                                xT[:, c, tt * P : (tt + 1) * P],
                                w_h[:, 2, c],
                                start=(c == 0),
                                stop=(c == CC - 1),
                            )
                    nc.vector.tensor_copy(
                        V_h[:, 4 * b + t4s.start : 4 * b + t4s.stop],
                        vps[:, t4s.start : t4s.stop],
                    )

            def att_block(h, b, filler=None):
                """Causal attention for head h, q block b -> yT[:, h, block].

                filler: list of zero-arg thunks (proj tiles); one is emitted
                after each y matmul to keep the PE fed while the ACT engine
                works through the exp chain.
                """
                qT_h, kT_h = qk_tiles[h]
                V_h = v_tiles[h]
                nch = 4 * (b + 1)
                bs = slice(b * 512, (b + 1) * 512)
                yps = ypsp.tile([P, 512], F32, name="yps")
                ds = dsp.tile([P, 512], BF16, name="ds")
                es = {}
                # last two (diagonal) chunks only touch the upper 256 q
                # columns; the lower half is fully masked — skip it
                qlo = {nch - 2: 256, nch - 1: 256}
                # software-pipelined: scores(c+1) issued before y(c)
                for c in range(nch + 1):
                    if c < nch:
                        lo = qlo.get(c, 0)
                        w_ = 512 - lo
                        qs = slice(b * 512 + lo, (b + 1) * 512)
                        st = stp.tile([P, 512], F32, name="st")
                        nc.tensor.matmul(
                            st[:, 0:w_],
                            kT_h[:, c * P : (c + 1) * P],
                            qT_h[:, qs],
                            start=True,
                            stop=True,
                        )
                        e_sb = ep.tile([P, 512], BF16, name="e_sb")
                        nc.scalar.activation(
                            e_sb[:, 0:w_], st[:, 0:w_], EXP, scale=SCALE
                        )
                        j = c - (nch - 4)
                        if j >= 0:
                            nc.vector.tensor_mul(
                                e_sb[:, 0:w_], e_sb[:, 0:w_],
                                masks[:, 384 - 128 * j + lo : 896 - 128 * j],
                            )
                        if c == 0:
                            nc.vector.tensor_copy(ds[:], e_sb[:])
                        else:
                            nc.vector.tensor_add(
                                ds[:, lo:512], ds[:, lo:512], e_sb[:, 0:w_]
                            )
                        es[c] = e_sb
                    if c >= 1:
                        cc = c - 1
                        lo = qlo.get(cc, 0)
                        nc.tensor.matmul(
                            yps[:, lo:512],
                            V_h[:, cc],
                            es.pop(cc)[:, 0 : 512 - lo],
                            start=(cc == 0),
                            stop=(cc == nch - 1),
                        )
                        if filler:
                            filler.pop(0)()
                # denominator: partition-sum + broadcast in one matmul
                bcs = stp.tile([P, 512], F32, name="st")
                nc.tensor.matmul(bcs[:], ones_bf[:], ds[:], start=True, stop=True)
                recip = rcp.tile([P, 512], BF16, name="recip")
                with nc.allow_low_precision(reason="bf16 softmax recip"):
                    nc.vector.reciprocal(recip[:], bcs[:])
                nc.vector.tensor_mul(yT[:, h, bs], yps[:], recip[:])

            def proj_tile(tt, nb):
                """One out tile: out[tt, nb] = sum_h yT[:,h,tt].T @ wp."""
                # rotate across 3 PSUM banks (2 in accp + the idle
                # vps bank) so copy-out latency never stalls the PE
                g = tt * NB + nb
                pool = vpsp if g % 3 == 2 else accp
                name = "vps" if g % 3 == 2 else "acc"
                ps3 = pool.tile([P, 512], F32, name=name)
                for hh in range(HL):
                    nc.tensor.matmul(
                        ps3[:],
                        yT[:, hh, tt * P : (tt + 1) * P],
                        wp_sb[:, hh, nb * 512 : (nb + 1) * 512],
                        start=(hh == 0),
                        stop=(hh == HL - 1),
                    )
                o_sb = op.tile([P, 512], BF16, name="o_sb")
                dst = out_d[
                    tt * P : (tt + 1) * P, nb * 512 : (nb + 1) * 512
                ]
                nc.scalar.copy(o_sb[:], ps3[:])
                nc.sync.dma_start(dst, o_sb[:])

            def proj_thunks(b):
                return [
                    (lambda tt=4 * b + t4, nb=nb: proj_tile(tt, nb))
                    for t4 in range(4)
                    for nb in range(NB)
                ]

            # --- fused pipeline ---
            for h in range(HL):
                if h + 1 < HL:
                    load_w(h + 1)  # prefetch next head's weights
                if h == 2:
                    nc.sync.dma_start(
                        wp_sb[:],
                        wp_d.rearrange("(h p) n -> p h n", p=P),
                    )
                for b in range(TB):
                    qkv_piece(h, b)
                    if h >= 1:
                        att_block(h - 1, b)
                    # last head: pull its attention forward one block so
                    # only one att block remains after the qkv rows
                    if h == HL - 1 and b >= 1:
                        att_block(HL - 1, b - 1)
            # tail: last att block with ready proj tiles as PE filler
            avail = []
            for b in range(TB - 1):
                avail.extend(proj_thunks(b))
            att_block(HL - 1, TB - 1, filler=avail)
            avail.extend(proj_thunks(TB - 1))
            for t in avail:
                t()

    nc.compile()
    return nc


def _rope_tables_T(T, head_dim):
    half = head_dim // 2
    inv_freq = 1.0 / (ROPE_THETA ** (np.arange(0, half, dtype=np.float64) / half))
    ang = np.arange(T, dtype=np.float64)[:, None] * inv_freq[None, :]  # [T, half]
    cos = np.concatenate([np.cos(ang), np.cos(ang)], axis=-1)  # [T, D]
    sin = np.concatenate([np.sin(ang), np.sin(ang)], axis=-1)
    return (
        np.ascontiguousarray(cos.T.astype(np.float32)),
        np.ascontiguousarray(sin.T.astype(np.float32)),
    )


def _make_masks():
    # masks[p, o] = (o >= p + 384); mask_j = masks[:, 384-128j : 896-128j]
    o = np.arange(896)[None, :]
    p = np.arange(P)[:, None]
    return (o >= p + 384).astype(np.float32)


_NC_CACHE = {}


def _get_nc(T, CIN, HL, COUT):
    key = (T, CIN, HL, COUT)
    if key not in _NC_CACHE:
        _NC_CACHE[key] = build_nc(T, CIN, HL, COUT)
    return _NC_CACHE[key]


def make_in_maps(x, w_attn, w_proj):
    import ml_dtypes

    bf16 = ml_dtypes.bfloat16
    x = np.asarray(x)
    w_attn = np.asarray(w_attn)
    w_proj = np.asarray(w_proj)
    B, T, C = x.shape
    HL = NUM_HEADS // 2  # 8 heads per core
    CL = HL * D  # 1024

    cosT, sinT = _rope_tables_T(T, D)
    cosT = cosT.astype(bf16)
    sinT = sinT.astype(bf16)
    masks = _make_masks().astype(bf16)

    in_maps = []
    for s in range(8):
        b, g = s // 2, s % 2
        # w layout [HL, 3, P, CC*D]: each (head, q/k/v) slab stored
        # partition-major so DMA descriptors are 4KB-contiguous
        CC = C // P
        qkv_cols = [
            w_attn[:, g * CL : (g + 1) * CL],
            w_attn[:, C + g * CL : C + (g + 1) * CL],
            w_attn[:, 2 * C + g * CL : 2 * C + (g + 1) * CL],
        ]
        w_shard = np.empty((HL, 3, P, CC * D), dtype=np.float32)
        for j, wj in enumerate(qkv_cols):
            # wj: [C, HL*D] -> per head slab [P, CC*D]
            s = wj.reshape(CC, P, HL, D).transpose(2, 1, 0, 3)  # [HL, P, CC, D]
            w_shard[:, j] = s.reshape(HL, P, CC * D)
        in_maps.append(
            {
                "xT": np.ascontiguousarray(x[b].T).astype(bf16),
                "w": np.ascontiguousarray(w_shard).astype(bf16),
                "wp": np.ascontiguousarray(
                    w_proj[g * CL : (g + 1) * CL, :]
                ).astype(bf16),
                "cosT": cosT,
                "sinT": sinT,
                "mask": masks,
            }
        )
    return in_maps


def combine(results, x_shape):
    B, T, C = x_shape
    out = np.empty((B, T, C), dtype=np.float32)
    for b in range(B):
        out[b] = results[2 * b]["out"].astype(np.float32) + results[
            2 * b + 1
        ]["out"].astype(np.float32)
    return out


def kernel(x, w_attn, w_proj):
    from concourse.bass_utils import run_bass_kernel_spmd

    x = np.asarray(x)
    B, T, C = x.shape  # 4, 2048, 2048
    HL = NUM_HEADS // 2

    nc = _get_nc(T, C, HL, C)
    in_maps = make_in_maps(x, w_attn, w_proj)
    res = run_bass_kernel_spmd(nc, in_maps, list(range(8)))
    return combine(res.results, (B, T, C))
